# revision 32
# baseline (speedup 1.0000x reference)
"""Trainium2 Bass kernel for nn_BasicS2ConvV2.

out[b,d,p,r] = sum_{c,k,a} Wfull[d,c,r,k,a] * x[b,c,k,p,a]
with Wfull gathered on host from the 36 free params (tiny), and the
31.4 GFLOP contraction run on 8 NeuronCores, data-parallel over b.

Per-core device problem: o[dr=192, p=4096] = WT[cka, dr]^T @ xs[cka, p]
with cka = 16*13*12 = 2496 padded to 2560 = 20 k-tiles of 128.

Host pre-layout makes every device DMA a fully sequential HBM stream:
  xs: [NPC, 128, KT, PC]  (one contiguous 2.6MB block per p-chunk tile)
  wt: [128, KT, DR]       (one contiguous block, loaded once)
  o : [NPC, DR, PC]       (one contiguous block per p-chunk output)
"""

import numpy as np
import ml_dtypes

B, C, KS, P, A = 8, 16, 13, 4096, 12
D, R = 16, 12
CKA = C * KS * A          # 2496
KT = 20                   # contraction tiles of 128 (2560 padded)
CKA_PAD = KT * 128
DR = D * R                # 192
PC = 1024                 # p-chunk width (two PSUM banks of fp32 per m-tile)
HB = 512                  # matmul moving free dim / PSUM bank width
NPC = P // PC             # 4

MMDT = "bf16"             # "bf16" | "f32r" | "f32"

_cache = {}


def _emit_body(nc, xs, wtile, o, io_dt, mm_dt, xpool, opool, pspool, reps,
               do_dma=True, do_mm=True, do_out=True, xt_static=None,
               only_m0=False):
    import concourse.mybir as mybir

    for pc in [pc for _ in range(reps) for pc in range(NPC)]:
        if do_dma:
            xt = xpool.tile([128, KT, PC], io_dt, tag="xt")
            # collapsed contiguous APs -> large DMA descriptors; two halves
            # so matmuls start after the first half lands
            xt_f = xt[:].rearrange("q t p -> q (t p)")
            xs_f = xs[pc].rearrange("q t p -> q (t p)")
            nc.scalar.dma_start(xt_f, xs_f)
        else:
            xt = xt_static
        if not do_mm:
            continue
        ps0 = [pspool.tile([128, HB], mybir.dt.float32, tag=f"ps0{h}",
                           name=f"ps0{h}") for h in range(PC // HB)]
        if only_m0:
            for h in range(PC // HB):
                for t in range(KT):
                    nc.tensor.matmul(
                        ps0[h][:], wtile[:, t, 0:128].bitcast(mm_dt),
                        xt[:, t, h * HB:(h + 1) * HB].bitcast(mm_dt),
                        start=(t == 0), stop=(t == KT - 1),
                    )
            continue
        ps1 = [pspool.tile([128, HB], mybir.dt.float32, tag=f"ps1{h}",
                           name=f"ps1{h}") for h in range(PC // HB)]
        # Same-PSUM-bank runs of 20 accumulating matmuls (bank switching
        # between consecutive matmuls causes PE micro-idles).
        for h in range(PC // HB):
            for t in range(KT):
                nc.tensor.matmul(
                    ps0[h][:], wtile[:, t, 0:128].bitcast(mm_dt),
                    xt[:, t, h * HB:(h + 1) * HB].bitcast(mm_dt),
                    start=(t == 0), stop=(t == KT - 1),
                )
            # m1 (dr 128:192, 64 wide): col-tiled pairs — even t in array
            # cols 0-63 -> psum partitions 0-63, odd t in cols 64-127 ->
            # partitions 64-127. Host adds the two half-sums.
            for t in range(KT):
                hf = t % 2
                nc.tensor.matmul(
                    ps1[h][64 * hf:64 * hf + 64, :],
                    wtile[:, t, 128:DR].bitcast(mm_dt),
                    xt[:, t, h * HB:(h + 1) * HB].bitcast(mm_dt),
                    start=(t == hf), stop=(t == KT - 2 + hf),
                    tile_position=(0, 64 * hf),
                )
        if not do_out:
            continue
        o0 = opool.tile([128, PC], mybir.dt.float32, tag="o0")
        o1 = opool.tile([128, PC], mybir.dt.float32, tag="o1")
        for h in range(PC // HB):
            nc.vector.tensor_copy(o0[:, h * HB:(h + 1) * HB], ps0[h][:])
            nc.vector.tensor_copy(o1[:, h * HB:(h + 1) * HB], ps1[h][:])
        nc.sync.dma_start(o[pc, 0:128, :], o0[:])
        nc.sync.dma_start(o[pc, 128:256, :], o1[:])


def _build_program(mmdt, reps=1, loop_n=0, do_dma=True, do_mm=True, do_out=True,
                   internal_io=False, only_m0=False):
    import concourse.bacc as bacc
    import concourse.mybir as mybir
    from concourse.tile import TileContext
    from contextlib import nullcontext

    io_dt = {
        "bf16": mybir.dt.bfloat16,
        "f32r": mybir.dt.float32,
        "f32": mybir.dt.float32,
    }[mmdt]
    mm_dt = {
        "bf16": mybir.dt.bfloat16,
        "f32r": mybir.dt.float32r,
        "f32": mybir.dt.float32,
    }[mmdt]

    nc = bacc.Bacc("TRN2", target_bir_lowering=False, debug=False)
    if internal_io:
        # Timing-probe build: no host I/O traffic; data is device garbage.
        xs = nc.dram_tensor("xs", [NPC, 128, KT, PC], io_dt).ap()
        wt = nc.dram_tensor("wt", [128, KT, DR], io_dt).ap()
        o = nc.dram_tensor("o", [NPC, DR + 64, PC], mybir.dt.float32).ap()
        dume = nc.declare_dram_parameter(
            "dume", [1, 8], mybir.dt.float32, isOutput=True)
    else:
        xs = nc.declare_dram_parameter(
            "xs", [NPC, 128, KT, PC], io_dt, isOutput=False)
        wt = nc.declare_dram_parameter(
            "wt", [128, KT, DR], io_dt, isOutput=False)
        o = nc.declare_dram_parameter(
            "o", [NPC, DR + 64, PC], mybir.dt.float32, isOutput=True)

    with TileContext(nc) as tc:
        with (
            tc.tile_pool(name="wpool", bufs=1) as wpool,
            tc.tile_pool(name="xpool", bufs=3) as xpool,
            tc.tile_pool(name="opool", bufs=3) as opool,
            tc.tile_pool(name="pspool", bufs=2, space="PSUM") as pspool,
        ):
            # All weights resident in one tile, one DMA (2560 x 192 < 1MB)
            wtile = wpool.tile([128, KT, DR], io_dt)
            nc.sync.dma_start(wtile[:], wt[:])

            xt_static = None
            if not do_dma:
                xt_static = wpool.tile([128, KT, PC], io_dt, tag="xt_static")
                nc.any.memset(xt_static[:], 0.25)

            loop_cm = tc.For_i(0, loop_n, 1) if loop_n else nullcontext()
            with loop_cm:
                _emit_body(nc, xs, wtile, o, io_dt, mm_dt,
                           xpool, opool, pspool, reps,
                           do_dma=do_dma, do_mm=do_mm, do_out=do_out,
                           xt_static=xt_static, only_m0=only_m0)

            if internal_io:
                dtile = opool.tile([1, 8], mybir.dt.float32, tag="dume")
                nc.any.memset(dtile[:], 1.0)
                nc.sync.dma_start(dume[:], dtile[:])

    nc.compile()
    return nc


def _get_program(mmdt):
    if mmdt not in _cache:
        _cache[mmdt] = _build_program(mmdt)
    return _cache[mmdt]


def _prep_np_dtype(mmdt):
    return ml_dtypes.bfloat16 if mmdt == "bf16" else np.float32


def _prep_inputs(x, W, idx_map, tivr, tir, mmdt):
    """Host prep: weight gather + relayout to sequential-DMA order."""
    np_dt = _prep_np_dtype(mmdt)

    Wm = W[:, :, idx_map].reshape(D, C, KS, A)
    Wfull = Wm[:, :, tivr[:, :, None], tir[:, None, :]]       # [d,c,r,k,a]
    WT = Wfull.transpose(1, 3, 4, 0, 2).reshape(CKA, DR)
    WT_pad = np.zeros((CKA_PAD, DR), dtype=np_dt)
    WT_pad[:CKA] = WT.astype(np_dt)
    # [2560, DR] -> [128(q), KT(t), DR]
    wt_q = np.ascontiguousarray(
        WT_pad.reshape(KT, 128, DR).transpose(1, 0, 2))

    # x[b,c,k,p,a] -> xs[b, (c,k,a), p] -> [b, NPC(pc), 128(q), KT(t), PC]
    xt = np.ascontiguousarray(x.transpose(0, 1, 2, 4, 3)).reshape(B, CKA, P)
    xs_pad = np.zeros((B, CKA_PAD, P), dtype=np_dt)
    xs_pad[:, :CKA] = xt.astype(np_dt)
    xs_q = np.ascontiguousarray(
        xs_pad.reshape(B, KT, 128, NPC, PC).transpose(0, 3, 2, 1, 4))
    return xs_q, wt_q


def kernel(x, W, idx_map, trace_idxv_rot, trace_idx_rot):
    from concourse.bass_utils import run_bass_kernel_spmd

    x = np.asarray(x)
    W = np.asarray(W, dtype=np.float32)
    idx_map = np.asarray(idx_map)
    tivr = np.asarray(trace_idxv_rot)
    tir = np.asarray(trace_idx_rot)

    xs_q, wt_q = _prep_inputs(x, W, idx_map, tivr, tir, MMDT)

    nc = _get_program(MMDT)
    in_maps = [{"xs": xs_q[b], "wt": wt_q} for b in range(B)]
    res = run_bass_kernel_spmd(nc, in_maps, list(range(B)))

    out = np.empty((B, D, P, R), dtype=np.float32)
    for b in range(B):
        # o[pc] rows: 0:128 = dr 0:128; 128:192 and 192:256 are the two
        # col-tiled half-sums of dr 128:192 -> add them on host.
        oraw = res.results[b]["o"].astype(np.float32)
        ob = np.concatenate(
            [oraw[:, 0:128, :], oraw[:, 128:192, :] + oraw[:, 192:256, :]],
            axis=1).reshape(NPC, D, R, PC)
        out[b] = ob.transpose(1, 0, 3, 2).reshape(D, P, R)
    return out
